# revision 16
# baseline (speedup 1.0000x reference)
"""GIN (3-layer) + global mean pool + MLP head on 8 Trainium2 NeuronCores.

Strategy: shard nodes (dst) across 8 cores. Host sorts edges by dst and packs
them into per-128-node-chunk gather lists. On device, per chunk: dma_gather
source rows from a replicated bf16 node-major table, build a one-hot (edge ->
local dst) matrix with a broadcast is_equal, and segment-sum via accumulating
matmuls into PSUM (feature-major [d, n] layout). Dense GIN MLP + training-mode
BatchNorm (global stats via AllReduce) computed feature-major; per layer the
local shard is transposed back to node-major, cast to bf16 and AllGathered
into the next gather table. Pooling = per-chunk transpose + matmul with a
graph-assignment matrix (scaled 1/count), AllReduce, then the readout MLP.
v2: bf16 gather table + one-hots (half the gather/collective bytes, bf16
matmuls), per-chunk gather tile counts (less padding than a global max).
"""
import sys
sys.path.insert(0, "/opt/trn_rl_repo")
import numpy as np
import ml_dtypes

import concourse.bass as bass
import concourse.bacc as bacc
import concourse.mybir as mybir
import concourse.tile as tile
from concourse.bass_utils import run_bass_kernel_spmd
from concourse.masks import make_identity

N = 50000
E = 500000
D = 128
L = 3
G = 64
OUT = 16
EPS = 1e-5
NCORES = 8
SH = N // NCORES          # 6250 nodes per core
NCH = (SH + 127) // 128   # 49 chunks
SHP = NCH * 128           # 6272 padded shard rows
TR = NCORES * SHP         # 50176 table rows
SPA = 3200                # rows per block in half A (chunks 0-24)
SPB = SHP - SPA           # 3072 rows per block in half B (chunks 25-48)
KA = SPA // 128           # 25 chunks in half A
NRA = NCORES * SPA        # 25600 table-A rows (< 32768 for int16 idx)
NRB = NCORES * SPB        # 24576 table-B rows
LASTC = SH - (NCH - 1) * 128  # 106 real cols in last chunk

F32 = mybir.dt.float32
BF16 = mybir.dt.bfloat16
I16 = mybir.dt.int16

_cache = {}


def _wrap_idx(vals, nslots, valid):
    """int row-indices -> dma_gather int16 layout [128, nslots*8].

    Slots [len(vals), valid) are dummy index 0 (gathered, zero one-hot);
    slots >= valid are -1 so descriptor emission stops there.
    """
    a = np.full(nslots * 128, -1, np.int16)
    a[: valid] = 0
    a[: len(vals)] = vals
    a = a.reshape(nslots * 8, 16).T  # [16, nslots*8]
    return np.tile(a, (8, 1))


def _build(TL, TH, mlo, mhi):
    TLM, THM = max(TL), max(TH)
    TM = max(TL[k] + TH[k] for k in range(NCH))
    nc = bacc.Bacc(None, num_devices=NCORES, num_swdge_queues=4,
                   dynamic_dma_scratch_size=131072)
    P = lambda n, s, dt=F32: nc.declare_dram_parameter(n, s, dt, isOutput=False)

    xtab_a = P("xtab_a", [NRA, D], BF16)
    xtab_b = P("xtab_b", [NRB, D], BF16)
    h0 = P("h0", [128, SHP])
    ilo = P("ilo", [128, NCH, TLM * 8], I16)
    ihi = P("ihi", [128, NCH, THM * 8], I16)
    dstl = P("dstl", [128, NCH, TM])
    gpool = P("gpool", [128, NCH, G])
    iota = P("iota", [128, 128])
    w1 = P("w1", [L, D, D])
    b1 = P("b1", [L, D, 1])
    bng = P("bng", [L, D, 1])
    bnb = P("bnb", [L, D, 1])
    w2 = P("w2", [L, D, D])
    b2 = P("b2", [L, D, 1])
    mw1 = P("mw1", [D, 64])
    mb1 = P("mb1", [64, 1])
    mw2 = P("mw2", [64, 32])
    mb2 = P("mb2", [32, 1])
    mw3 = P("mw3", [32, OUT])
    mb3 = P("mb3", [OUT, 1])
    out = nc.declare_dram_parameter("out", [OUT, G], F32, isOutput=True)

    znm = nc.dram_tensor("znm", [SHP, D], BF16)
    tbls_a = [xtab_a] + [
        nc.dram_tensor(f"tbla{l}", [NRA, D], BF16, addr_space="Shared")
        for l in (1, 2)
    ]
    tbls_b = [xtab_b] + [
        nc.dram_tensor(f"tblb{l}", [NRB, D], BF16, addr_space="Shared")
        for l in (1, 2)
    ]
    ccin = nc.dram_tensor("ccin", [128, 2], F32)
    ccout = nc.dram_tensor("ccout", [128, 2], F32, addr_space="Shared")
    plin = nc.dram_tensor("plin", [128, G], F32)
    plout = nc.dram_tensor("plout", [128, G], F32, addr_space="Shared")

    RG = [list(range(NCORES))]
    AF = mybir.ActivationFunctionType
    X = mybir.AxisListType.X

    with tile.TileContext(nc) as tc:
        with tc.tile_pool(name="const", bufs=1) as cp, \
             tc.tile_pool(name="big", bufs=1) as bp, \
             tc.tile_pool(name="sb", bufs=3) as sb, \
             tc.tile_pool(name="st", bufs=2) as stp, \
             tc.tile_pool(name="pp", bufs=2, space="PSUM") as pp, \
             tc.tile_pool(name="pp1", bufs=1, space="PSUM") as pp1:

            iota_t = cp.tile([128, 1, 128], F32, tag="iota")
            nc.sync.dma_start(out=iota_t[:], in_=iota[:, None, :])
            ident = cp.tile([128, 128], F32, tag="ident")
            make_identity(nc, ident[:])
            zz = cp.tile([32, 128], BF16, tag="zz")
            nc.vector.memset(zz[:], 0.0)
            epst = cp.tile([128, 1], F32, tag="epst")
            nc.vector.memset(epst[:], EPS)

            w1_t, w2_t, b1_t, b2_t, bng_t, bnb_t = [], [], [], [], [], []
            for l in range(L):
                a = cp.tile([D, D], F32, tag=f"w1_{l}")
                nc.sync.dma_start(out=a[:], in_=w1[l])
                w1_t.append(a)
                a = cp.tile([D, D], F32, tag=f"w2_{l}")
                nc.sync.dma_start(out=a[:], in_=w2[l])
                w2_t.append(a)
                for tlist, src, nm in ((b1_t, b1, "b1"), (b2_t, b2, "b2"),
                                       (bng_t, bng, "bng"), (bnb_t, bnb, "bnb")):
                    a = cp.tile([D, 1], F32, tag=f"{nm}_{l}")
                    nc.sync.dma_start(out=a[:], in_=src[l])
                    tlist.append(a)
            mw1_t = cp.tile([D, 64], F32, tag="mw1")
            nc.sync.dma_start(out=mw1_t[:], in_=mw1[:])
            mw2_t = cp.tile([64, 32], F32, tag="mw2")
            nc.sync.dma_start(out=mw2_t[:], in_=mw2[:])
            mw3_t = cp.tile([32, OUT], F32, tag="mw3")
            nc.sync.dma_start(out=mw3_t[:], in_=mw3[:])
            mb1_t = cp.tile([64, 1], F32, tag="mb1")
            nc.sync.dma_start(out=mb1_t[:], in_=mb1[:])
            mb2_t = cp.tile([32, 1], F32, tag="mb2")
            nc.sync.dma_start(out=mb2_t[:], in_=mb2[:])
            mb3_t = cp.tile([OUT, 1], F32, tag="mb3")
            nc.sync.dma_start(out=mb3_t[:], in_=mb3[:])

            # pre-zero the rotating gather slots: rows past num_idxs_reg are
            # never written by the gather and must not hold NaN bit patterns
            for _ in range(3):
                t = sb.tile([128, TLM, D], BF16, tag="glo")
                nc.vector.memset(t[:], 0.0)
                t = sb.tile([128, THM, D], BF16, tag="ghi")
                nc.vector.memset(t[:], 0.0)

            ilo_t = cp.tile([128, NCH, TLM * 8], I16, tag="ilo")
            nc.sync.dma_start(out=ilo_t[:], in_=ilo[:])
            ihi_t = cp.tile([128, NCH, THM * 8], I16, tag="ihi")
            nc.sync.dma_start(out=ihi_t[:], in_=ihi[:])
            dl_t = cp.tile([128, NCH, TM], F32, tag="dlt")
            nc.sync.dma_start(out=dl_t[:], in_=dstl[:])
            h_fm = bp.tile([128, SHP], F32, tag="h")
            nc.sync.dma_start(out=h_fm[:], in_=h0[:])
            z1_fm = bp.tile([128, SHP], F32, tag="z1")
            stats_s = bp.tile([128, 64], F32, tag="ss")
            stats_q = bp.tile([128, 64], F32, tag="sq")
            pooled_ps = pp1.tile([128, G], F32, tag="pooled")

            for l in range(L):
                table_a, table_b = tbls_a[l], tbls_b[l]
                # ---- pass A: aggregate + first linear + BN stats ----
                for k in range(NCH):
                    ks = slice(k * 128, (k + 1) * 128)
                    ncols = LASTC if k == NCH - 1 else 128
                    tlk, thk = TL[k], TH[k]
                    tk = tlk + thk
                    glo = sb.tile([128, TLM, D], BF16, tag="glo")
                    nc.gpsimd.dma_gather(glo[:, :tlk, :], table_a[:, :],
                                         ilo_t[:, k, :tlk * 8],
                                         tlk * 128, int(mlo[k]), D,
                                         queue_num=(k % 2) * 2)
                    ghi = sb.tile([128, THM, D], BF16, tag="ghi")
                    nc.gpsimd.dma_gather(ghi[:, :thk, :], table_b[:, :],
                                         ihi_t[:, k, :thk * 8],
                                         thk * 128, int(mhi[k]), D,
                                         queue_num=(k % 2) * 2 + 1)
                    oh = sb.tile([128, TM, 128], BF16, tag="oh")
                    nc.vector.tensor_tensor(
                        out=oh[:, :tk, :],
                        in0=dl_t[:, k, :tk, None].to_broadcast([128, tk, 128]),
                        in1=iota_t[:].to_broadcast([128, tk, 128]),
                        op=mybir.AluOpType.is_equal)
                    ps_agg = pp.tile([128, 128], F32, tag="agg")
                    for t in range(tk):
                        g_sub = glo[:, t, :] if t < tlk else ghi[:, t - tlk, :]
                        nc.tensor.matmul(ps_agg[:], g_sub, oh[:, t, :],
                                         start=(t == 0), stop=(t == tk - 1))
                    z0 = sb.tile([128, 128], F32, tag="z0")
                    nc.vector.tensor_add(out=z0[:], in0=ps_agg[:], in1=h_fm[:, ks])
                    ps_mm = pp.tile([128, 128], F32, tag="mm")
                    nc.tensor.matmul(ps_mm[:], w1_t[l][:], z0[:], start=True, stop=True)
                    nc.vector.tensor_scalar_add(out=z1_fm[:, ks], in0=ps_mm[:],
                                                scalar1=b1_t[l][:])
                    zsl = z1_fm[:, k * 128: k * 128 + ncols]
                    nc.vector.reduce_sum(out=stats_s[:, k:k + 1], in_=zsl, axis=X)
                    sq = sb.tile([128, 128], F32, tag="sqt")
                    nc.vector.tensor_tensor(out=sq[:, :ncols], in0=zsl, in1=zsl,
                                            op=mybir.AluOpType.mult)
                    nc.vector.reduce_sum(out=stats_q[:, k:k + 1], in_=sq[:, :ncols],
                                         axis=X)
                # ---- BN stats: AllReduce + scale/shift ----
                s_tot = stp.tile([128, 1], F32, tag="s_tot")
                nc.vector.reduce_sum(out=s_tot[:], in_=stats_s[:, :NCH], axis=X)
                q_tot = stp.tile([128, 1], F32, tag="q_tot")
                nc.vector.reduce_sum(out=q_tot[:], in_=stats_q[:, :NCH], axis=X)
                cc = stp.tile([128, 2], F32, tag="cc")
                nc.vector.tensor_copy(out=cc[:, 0:1], in_=s_tot[:])
                nc.vector.tensor_copy(out=cc[:, 1:2], in_=q_tot[:])
                nc.sync.dma_start(out=ccin[:], in_=cc[:])
                nc.gpsimd.collective_compute(
                    "AllReduce", mybir.AluOpType.add, replica_groups=RG,
                    ins=[ccin[:]], outs=[ccout[:]])
                stg = stp.tile([128, 2], F32, tag="stg")
                nc.sync.dma_start(out=stg[:], in_=ccout[:])
                mean = stp.tile([128, 1], F32, tag="mean")
                nc.vector.tensor_scalar_mul(out=mean[:], in0=stg[:, 0:1], scalar1=1.0 / N)
                ex2 = stp.tile([128, 1], F32, tag="ex2")
                nc.vector.tensor_scalar_mul(out=ex2[:], in0=stg[:, 1:2], scalar1=1.0 / N)
                msq = stp.tile([128, 1], F32, tag="msq")
                nc.vector.tensor_tensor(out=msq[:], in0=mean[:], in1=mean[:],
                                        op=mybir.AluOpType.mult)
                var = stp.tile([128, 1], F32, tag="var")
                nc.vector.tensor_tensor(out=var[:], in0=ex2[:], in1=msq[:],
                                        op=mybir.AluOpType.subtract)
                sqv = stp.tile([128, 1], F32, tag="sqv")
                nc.scalar.activation(sqv[:], var[:], AF.Sqrt, bias=epst[:])
                rstd = stp.tile([128, 1], F32, tag="rstd")
                nc.vector.reciprocal(out=rstd[:], in_=sqv[:])
                scale = stp.tile([128, 1], F32, tag="scale")
                nc.vector.tensor_tensor(out=scale[:], in0=bng_t[l][:], in1=rstd[:],
                                        op=mybir.AluOpType.mult)
                mscl = stp.tile([128, 1], F32, tag="mscl")
                nc.vector.tensor_tensor(out=mscl[:], in0=mean[:], in1=scale[:],
                                        op=mybir.AluOpType.mult)
                shift = stp.tile([128, 1], F32, tag="shift")
                nc.vector.tensor_tensor(out=shift[:], in0=bnb_t[l][:], in1=mscl[:],
                                        op=mybir.AluOpType.subtract)
                # ---- pass B: BN+relu, second linear, transpose out ----
                for k in range(NCH):
                    ks = slice(k * 128, (k + 1) * 128)
                    z1n = sb.tile([128, 128], F32, tag="z1n")
                    nc.scalar.activation(z1n[:], z1_fm[:, ks], AF.Relu,
                                         bias=shift[:], scale=scale[:])
                    ps_mm = pp.tile([128, 128], F32, tag="mm")
                    nc.tensor.matmul(ps_mm[:], w2_t[l][:], z1n[:], start=True, stop=True)
                    if l < L - 1:
                        nc.scalar.activation(h_fm[:, ks], ps_mm[:], AF.Relu,
                                             bias=b2_t[l][:])
                    else:
                        nc.vector.tensor_scalar_add(out=h_fm[:, ks], in0=ps_mm[:],
                                                    scalar1=b2_t[l][:])
                    ps_tr = pp.tile([128, 128], F32, tag="tr")
                    nc.tensor.transpose(out=ps_tr[:], in_=h_fm[:, ks], identity=ident[:])
                    if l < L - 1:
                        znm_t = sb.tile([128, 128], BF16, tag="znm")
                        nc.vector.tensor_copy(out=znm_t[:], in_=ps_tr[:])
                        nc.sync.dma_start(out=znm[ks, :], in_=znm_t[:])
                        if k == KA - 1:
                            nc.gpsimd.collective_compute(
                                "AllGather", mybir.AluOpType.bypass,
                                replica_groups=RG,
                                ins=[znm[:SPA]], outs=[tbls_a[l + 1][:]])
                    else:
                        znm_f = sb.tile([128, 128], F32, tag="znmf")
                        nc.vector.tensor_copy(out=znm_f[:], in_=ps_tr[:])
                        gp = sb.tile([128, G], F32, tag="gp")
                        nc.sync.dma_start(out=gp[:], in_=gpool[:, k, :])
                        nc.tensor.matmul(pooled_ps[:], znm_f[:], gp[:],
                                         start=(k == 0), stop=(k == NCH - 1))
                if l < L - 1:
                    nc.sync.dma_start(out=znm[SH:SHP, :], in_=zz[:SHP - SH, :])
                    nc.gpsimd.collective_compute(
                        "AllGather", mybir.AluOpType.bypass, replica_groups=RG,
                        ins=[znm[SPA:]], outs=[tbls_b[l + 1][:]])

            # ---- pooling AllReduce + readout MLP ----
            plt = stp.tile([128, G], F32, tag="plt")
            nc.vector.tensor_copy(out=plt[:], in_=pooled_ps[:])
            nc.sync.dma_start(out=plin[:], in_=plt[:])
            nc.gpsimd.collective_compute(
                "AllReduce", mybir.AluOpType.add, replica_groups=RG,
                ins=[plin[:]], outs=[plout[:]])
            pl = stp.tile([128, G], F32, tag="pl")
            nc.sync.dma_start(out=pl[:], in_=plout[:])
            ps_r = pp.tile([64, G], F32, tag="mm")
            nc.tensor.matmul(ps_r[:], mw1_t[:], pl[:], start=True, stop=True)
            r1 = stp.tile([64, G], F32, tag="r1")
            nc.scalar.activation(r1[:], ps_r[:], AF.Relu, bias=mb1_t[:])
            ps_r2 = pp.tile([32, G], F32, tag="tr")
            nc.tensor.matmul(ps_r2[:], mw2_t[:], r1[:], start=True, stop=True)
            r2 = stp.tile([32, G], F32, tag="r2")
            nc.scalar.activation(r2[:], ps_r2[:], AF.Relu, bias=mb2_t[:])
            ps_r3 = pp.tile([OUT, G], F32, tag="agg")
            nc.tensor.matmul(ps_r3[:], mw3_t[:], r2[:], start=True, stop=True)
            ot = stp.tile([OUT, G], F32, tag="ot")
            nc.vector.tensor_scalar_add(out=ot[:], in0=ps_r3[:], scalar1=mb3_t[:])
            nc.sync.dma_start(out=out[:], in_=ot[:])
    nc.compile()
    return nc


def _prep(x, edge_index, edge_attr, batch, lin1_w, lin1_b, bn_g, bn_b,
          lin2_w, lin2_b, mlp_w1, mlp_b1, mlp_w2, mlp_b2, mlp_w3, mlp_b3):
    x = np.asarray(x, np.float32)
    ei = np.asarray(edge_index).astype(np.int64)
    batch = np.asarray(batch).astype(np.int64)
    src, dst = ei[0], ei[1]

    sblk = src // SH
    srow = src % SH
    core = dst // SH
    chunk = (dst % SH) // 128
    dstloc = (dst % SH) % 128
    is_a = srow < SPA
    tidx = np.where(is_a, sblk * SPA + srow,
                    sblk * SPB + (srow - SPA)).astype(np.int64)
    key = (core * NCH + chunk).astype(np.int64)

    order = np.argsort(key, kind="stable")
    key_s, tidx_s, dstloc_s = key[order], tidx[order], dstloc[order]
    is_lo = is_a[order]
    counts = np.bincount(key_s, minlength=NCORES * NCH)
    lo_counts = np.bincount(key_s[is_lo], minlength=NCORES * NCH)
    hi_counts = counts - lo_counts
    # per-chunk max valid count across cores (compile-time num_idxs_reg)
    mlo = np.maximum(lo_counts.reshape(NCORES, NCH).max(0), 1)
    mhi = np.maximum(hi_counts.reshape(NCORES, NCH).max(0), 1)
    TL = tuple(int(-(-int(m) // 128)) for m in mlo)
    TH = tuple(int(-(-int(m) // 128)) for m in mhi)
    TLM, THM = max(TL), max(TH)
    TM = max(TL[k] + TH[k] for k in range(NCH))

    starts = np.zeros(NCORES * NCH + 1, np.int64)
    np.cumsum(counts, out=starts[1:])

    ilo_a = np.full((NCORES, NCH, 128, TLM * 8), -1, np.int16)
    ihi_a = np.full((NCORES, NCH, 128, THM * 8), -1, np.int16)
    dstl_a = np.full((NCORES, NCH, 128, TM), -1.0, np.float32)
    # device layout is [128, NCH, W]; transposed at the end of _prep
    for c in range(NCORES):
        for k in range(NCH):
            kk = c * NCH + k
            s, e = starts[kk], starts[kk + 1]
            ti, dl, lo = tidx_s[s:e], dstloc_s[s:e], is_lo[s:e]
            tlo, dlo = ti[lo], dl[lo]
            thi, dhi = ti[~lo], dl[~lo]
            ilo_a[c, k, :, :TL[k] * 8] = _wrap_idx(tlo, TL[k], int(mlo[k]))
            ihi_a[c, k, :, :TH[k] * 8] = _wrap_idx(thi, TH[k], int(mhi[k]))
            nlo, nhi = len(tlo), len(thi)
            if nlo:
                dstl_a[c, k, np.arange(nlo) % 128, np.arange(nlo) // 128] = dlo
            if nhi:
                dstl_a[c, k, np.arange(nhi) % 128,
                       TL[k] + np.arange(nhi) // 128] = dhi

    xt = np.zeros((NCORES, SHP, D), np.float32)
    h0_a = np.zeros((NCORES, 128, SHP), np.float32)
    for c in range(NCORES):
        xs = x[c * SH:(c + 1) * SH]
        xt[c, :SH] = xs
        h0_a[c, :, :SH] = xs.T
    xtab_a = xt[:, :SPA].reshape(NRA, D).astype(ml_dtypes.bfloat16)
    xtab_b = xt[:, SPA:].reshape(NRB, D).astype(ml_dtypes.bfloat16)

    cnts = np.bincount(batch, minlength=G).astype(np.float32)
    inv = 1.0 / np.maximum(cnts, 1.0)
    gpool_a = np.zeros((NCORES, NCH, 128, G), np.float32)
    for c in range(NCORES):
        b = batch[c * SH:(c + 1) * SH]
        n = len(b)
        p = np.arange(n)
        gpool_a[c, p // 128 % NCH + 0, p % 128, b] = inv[b]
    iota_a = np.tile(np.arange(128, dtype=np.float32)[None, :], (128, 1))

    common = {
        "xtab_a": xtab_a, "xtab_b": xtab_b, "iota": iota_a,
        "w1": np.asarray(lin1_w, np.float32),
        "b1": np.asarray(lin1_b, np.float32).reshape(L, D, 1),
        "bng": np.asarray(bn_g, np.float32).reshape(L, D, 1),
        "bnb": np.asarray(bn_b, np.float32).reshape(L, D, 1),
        "w2": np.asarray(lin2_w, np.float32),
        "b2": np.asarray(lin2_b, np.float32).reshape(L, D, 1),
        "mw1": np.asarray(mlp_w1, np.float32),
        "mb1": np.asarray(mlp_b1, np.float32).reshape(64, 1),
        "mw2": np.asarray(mlp_w2, np.float32),
        "mb2": np.asarray(mlp_b2, np.float32).reshape(32, 1),
        "mw3": np.asarray(mlp_w3, np.float32),
        "mb3": np.asarray(mlp_b3, np.float32).reshape(OUT, 1),
    }
    in_maps = []
    for c in range(NCORES):
        m = dict(common)
        m["h0"] = h0_a[c]
        m["ilo"] = np.ascontiguousarray(ilo_a[c].transpose(1, 0, 2))
        m["ihi"] = np.ascontiguousarray(ihi_a[c].transpose(1, 0, 2))
        m["dstl"] = np.ascontiguousarray(dstl_a[c].transpose(1, 0, 2))
        m["gpool"] = np.ascontiguousarray(gpool_a[c].transpose(1, 0, 2))
        in_maps.append(m)
    return TL, TH, tuple(int(v) for v in mlo), tuple(int(v) for v in mhi), in_maps


def kernel(**inputs):
    TL, TH, mlo, mhi, in_maps = _prep(**inputs)
    kk = (TL, TH, mlo, mhi)
    if kk not in _cache:
        _cache[kk] = _build(TL, TH, mlo, mhi)
    r = run_bass_kernel_spmd(_cache[kk], in_maps, list(range(NCORES)))
    return np.ascontiguousarray(np.asarray(r.results[0]["out"]).T.astype(np.float32))
